# revision 24
# baseline (speedup 1.0000x reference)
"""Trainium2 Bass kernel for nn_AttentionBlock (B=4, H=W=64, C=256, D=32).

Sharding: 8 shards = 4 samples x 2 query-halves. Each core gets the full
sample (rows reordered so its 2048 query rows come first), computes K and
the fused V@Wo projection for all 4096 keys, and attention + residual for
its 2048 queries. No collectives needed.

Key structure (v4):
  - x is transposed on the host (layout prep), so no on-device transpose
    phase: xT arrives channel-major [128, 2, 4096].
  - wo is folded into the value projection on the host: W = x @ (wv@wo)
    (+ bv@wo). Attention output then IS the final projection:
    out = (exp(S) @ W) / denom + x_residual. No epilogue transpose or
    output-projection matmuls.
  - Scores (contraction D=32): q/k are replicated across the 4 partition
    bands (via column-replicated wq/wk), and the score matmul contracts
    over the FULL 128 partitions, computing 4x the dot product (the 1/4
    is folded into the host-side wq scale). Cost of a matmul is its
    free-dim column count (contraction-depth independent), so the
    redundancy is free; in exchange the PE never changes tile mode
    between score and attend matmuls (no array-drain penalties).
  - Softmax denominator via a ones-column appended to W (free dim 257).
  - Score PSUM [128, 1024] x 3 buffers (6 banks) so exp of superstep u
    overlaps score/attend matmuls of u+1, u+2 with no bank collisions.

Self-contained: hardcodes shapes, imports only /opt/trn_rl_repo concourse.
"""

import sys

if "/opt/trn_rl_repo" not in sys.path:
    sys.path.insert(0, "/opt/trn_rl_repo")

import numpy as np
import ml_dtypes

BF16 = ml_dtypes.bfloat16

# Problem constants
B, HH, WW, C = 4, 64, 64, 256
D = 32
N = HH * WW          # 4096 keys per sample
NQ = N // 2          # 2048 queries per core
NCORES = 8
KC = N // 128        # 32 key chunks

_compiled_cache = {}


def _build(use_bias: bool):
    from contextlib import ExitStack
    from concourse import bacc, tile, mybir

    f32 = mybir.dt.float32
    bf = mybir.dt.bfloat16
    f8 = mybir.dt.float8e4
    DR = mybir.MatmulPerfMode.DoubleRow

    nc = bacc.Bacc("TRN2", target_bir_lowering=False, debug=False, num_devices=NCORES)

    xT_d = nc.dram_tensor("xT", [128, 2, N], bf, kind="ExternalInput")
    xq32_d = nc.dram_tensor("xq32", [NQ, C], f32, kind="ExternalInput")
    # single blob: [wq0|wq1|wk0|wk1|wvo0|wvo1] so one DMA loads all weights
    wb_d = nc.dram_tensor("wblob", [128, 1024], bf, kind="ExternalInput")
    wbias_d = (
        nc.dram_tensor("wbias", [1, 512], bf, kind="ExternalInput")
        if use_bias
        else None
    )
    out_d = nc.dram_tensor("out", [NQ, C], f32, kind="ExternalOutput")

    Exp = mybir.ActivationFunctionType.Exp
    Add = mybir.AluOpType.add
    Mult = mybir.AluOpType.mult

    with tile.TileContext(nc) as tc:
        with ExitStack() as ctx:
            const = ctx.enter_context(tc.tile_pool(name="const", bufs=1))
            big = ctx.enter_context(tc.tile_pool(name="big", bufs=1))
            xbp = ctx.enter_context(tc.tile_pool(name="xbp", bufs=3))
            expp = ctx.enter_context(tc.tile_pool(name="expp", bufs=4))
            small = ctx.enter_context(tc.tile_pool(name="small", bufs=2))
            # PSUM: scores 2x2 banks + phase-B 2 banks + pa 2 banks = 8.
            ps_sc = ctx.enter_context(tc.tile_pool(name="ps_sc", bufs=2, space="PSUM"))
            ps_b = ctx.enter_context(tc.tile_pool(name="ps_b", bufs=2, space="PSUM"))
            ps_pa = ctx.enter_context(tc.tile_pool(name="ps_pa", bufs=2, space="PSUM"))

            # ---- weights (one blob DMA) ----
            wall = const.tile([128, 1024], bf, tag="wall")
            nc.sync.dma_start(out=wall[:], in_=wb_d[:])
            wq0 = wall[:, 0:128]
            wq1 = wall[:, 128:256]
            wk0 = wall[:, 256:384]
            wk1 = wall[:, 384:512]
            wvo0 = wall[:, 512:768]
            wvo1 = wall[:, 768:1024]
            if use_bias:
                ones_row = const.tile([1, 512], bf, tag="ones_row")
                nc.gpsimd.memset(ones_row[:], 1.0)
                wbias = const.tile([1, 512], bf, tag="wbias")
                nc.sync.dma_start(out=wbias[:], in_=wbias_d[:])
                wqb = wbias[:, 0:128]
                wkb = wbias[:, 128:256]
                wvob = wbias[:, 256:512]

            # Persistent SBUF: qT/kT replicated across the 4 partition bands
            # (so full-128-contraction score matmuls compute 4x the dot
            # product), and W (= V@Wo) rows with a ones column at 256 for the
            # softmax denominator.
            qT4 = big.tile([128, NQ], bf, tag="qT4")
            kT4 = big.tile([128, N], bf, tag="kT4")
            wsb = big.tile([128, 16, 2, 272], f8, tag="wsb")
            nc.vector.memset(wsb[:, :, :, 256:257], 1.0)
            expbias = const.tile([128, 1], f32, tag="expbias")
            nc.vector.memset(expbias[:], -2.0)

            # ---- phase B: DMA xT chunks, q/k proj (LDW-amortized), W proj ----
            xbs = []
            for s in range(8):
                xb = xbp.tile([128, 2, 512], bf, tag="xb", bufs=8)
                nc.sync.dma_start(out=xb[:], in_=xT_d[:, :, 512 * s : 512 * s + 512])
                xbs.append(xb)

            def qk_chunk(dst, w0, w1, wb, s, nm):
                p = ps_b.tile([128, 512], f32, tag="x", name=f"{nm}{s}")
                nc.tensor.matmul(p[:], w0, xbs[s][:, 0, :], start=True, stop=False)
                nc.tensor.matmul(p[:], w1, xbs[s][:, 1, :], start=False, stop=not use_bias)
                if use_bias:
                    nc.tensor.matmul(p[:], wb, ones_row[:], start=False, stop=True)
                nc.vector.tensor_copy(dst[:, 512 * s : 512 * s + 512], p[:])

            def w_pass(s):
                # W = x @ (wv@wo): 4 key chunks of 128 per xb, 2 chunks per psum
                for half in range(2):
                    pw = ps_b.tile([128, 512], f32, tag="x", name=f"pw{s}_{half}")
                    for j in range(2):
                        off = 128 * (2 * half + j)
                        nc.tensor.matmul(pw[:, 256 * j : 256 * j + 256], xbs[s][:, 0, off : off + 128], wvo0, start=True, stop=False)
                        nc.tensor.matmul(pw[:, 256 * j : 256 * j + 256], xbs[s][:, 1, off : off + 128], wvo1, start=False, stop=not use_bias)
                        if use_bias:
                            nc.tensor.matmul(pw[:, 256 * j : 256 * j + 256], ones_row[:, 0:128], wvob, start=False, stop=True)
                    P = 2 * s + half
                    # evacuate on vector engine (scalar stays free for exps)
                    nc.vector.tensor_copy(wsb[:, P, :, 0:256], pw[:])

            # ---- phases B + C/D, software-pipelined ----
            # Phase B is emitted chunk-major and INTERLEAVED with group 0's
            # supersteps: superstep (0, u) only needs q chunk 0 and k/W of
            # chunk u, so the PE alternates projection and attention work
            # during the ramp and the scalar engine starts exp-ing early.
            # Superstep u of group g covers key chunks m = 4u + j. The score
            # matmul contracts over all 128 partitions (4 replicated bands of
            # q/k), computing 4x the true dot product; 1/4 is folded into the
            # host-side wq scale. Score psum [128, 1024] x bufs=3 (6 banks).

            def epilogue(qb, pa_t):
                rec = small.tile([128, 1], f32, tag="rec")
                nc.vector.reciprocal(rec[:], pa_t[:, 256:257])
                xq = small.tile([128, 256], f32, tag="xq", bufs=3)
                nc.sync.dma_start(out=xq[:], in_=xq32_d[128 * qb : 128 * qb + 128, :])
                sc = small.tile([128, 256], f32, tag="sc2")
                nc.vector.tensor_scalar(sc[:], pa_t[:, 0:256], rec[:], None, Mult)
                ot = small.tile([128, 256], f32, tag="ot", bufs=3)
                nc.vector.tensor_tensor(ot[:], sc[:], xq[:], Add)
                nc.sync.dma_start(out=out_d[128 * qb : 128 * qb + 128, :], in_=ot[:])

            pa_tiles = {}
            prev = None  # (et, g, u)
            NSS = 8 * 8  # 8 groups x 8 supersteps
            for idx in range(NSS + 1):
                if idx < 8:
                    # phase B chunk idx feeds superstep (0, idx)
                    s = idx
                    if s < 4:
                        qk_chunk(qT4, wq0, wq1, wqb if use_bias else None, s, "pq")
                    qk_chunk(kT4, wk0, wk1, wkb if use_bias else None, s, "pk")
                    w_pass(s)
                if idx < NSS:
                    g, u = divmod(idx, 8)
                    if u == 0:
                        pa_tiles[2 * g] = ps_pa.tile([128, 512], f32, tag="pa", name=f"pa{2 * g}")
                        pa_tiles[2 * g + 1] = ps_pa.tile([128, 512], f32, tag="pa", name=f"pa{2 * g + 1}")
                    pst = ps_sc.tile([128, 1024], f32, tag="sc")
                    for j in range(4):
                        m = 4 * u + j
                        nc.tensor.matmul(
                            pst[:, 256 * j : 256 * j + 256],
                            kT4[:, 128 * m : 128 * m + 128],
                            qT4[:, 256 * g : 256 * g + 256],
                            start=True,
                            stop=True,
                        )
                    # exp(s - 2): constant shift keeps exp within fp8-e4m3
                    # range (max score ~7 -> e^5 = 148 << 448; the tail below
                    # s = -4.2 flushes to zero, ~1e-7 of softmax mass). The
                    # ones-column denominator sees the same shift, so the
                    # normalized ratio is exact.
                    et = expp.tile([128, 2, 2, 256], f8, tag="e")
                    nc.scalar.activation(et[:], pst[:], Exp, bias=expbias[:])
                # attend with previous superstep's exp tile (keeps PE busy during exp)
                if prev is not None:
                    et_p, g_p, u_p = prev
                    for jp in range(2):
                        P = 2 * u_p + jp  # wsb pair: key chunks 2P, 2P+1
                        for h in range(2):
                            nc.tensor.matmul(
                                pa_tiles[2 * g_p + h][:, 0:257],
                                et_p[:, jp, :, 128 * h : 128 * h + 128],
                                wsb[:, P, :, 0:257],
                                start=(P == 0),
                                stop=(P == 15),
                                perf_mode=DR,
                            )
                    if u_p == 7:
                        for h in range(2):
                            epilogue(2 * g_p + h, pa_tiles[2 * g_p + h])
                            del pa_tiles[2 * g_p + h]
                if idx < NSS:
                    prev = (et, g, u)

    nc.compile()
    return nc


def _get_compiled(use_bias: bool):
    key = bool(use_bias)
    if key not in _compiled_cache:
        _compiled_cache[key] = _build(use_bias)
    return _compiled_cache[key]


def _prep(x, wq, bq, wk, bk, wv, bv, wo, bo):
    xf = np.ascontiguousarray(np.asarray(x, dtype=np.float32)).reshape(B, N, C)
    wq = np.asarray(wq, np.float32)
    bq = np.asarray(bq, np.float32)
    wk = np.asarray(wk, np.float32)
    bk = np.asarray(bk, np.float32)
    wv = np.asarray(wv, np.float32)
    bv = np.asarray(bv, np.float32)
    wo = np.asarray(wo, np.float32)
    bo = np.asarray(bo, np.float32)

    use_bias = not (
        np.all(bq == 0) and np.all(bk == 0) and np.all(bv == 0) and np.all(bo == 0)
    )

    # fold softmax scale into q, plus 1/4 because the score matmul contracts
    # over 4 replicated partition bands (summing the dot product 4x)
    scale = np.float32(1.0 / (4.0 * np.sqrt(np.float32(D))))
    wq_rep = np.tile(wq * scale, (1, 4)).astype(BF16)  # [256, 128]
    wk_rep = np.tile(wk, (1, 4)).astype(BF16)
    # fold wo into the value projection: W = x @ (wv@wo) + bv@wo
    wvo = (wv @ wo).astype(BF16)
    wblob = np.ascontiguousarray(
        np.concatenate(
            [wq_rep[0:128], wq_rep[128:256], wk_rep[0:128], wk_rep[128:256],
             wvo[0:128], wvo[128:256]],
            axis=1,
        )
    )  # [128, 1024]
    wbias = np.ascontiguousarray(
        np.concatenate(
            [np.tile(bq * scale, 4), np.tile(bk, 4), bv @ wo], 0
        )[None, :]
    ).astype(BF16)  # [1, 512]

    in_maps = []
    for core in range(NCORES):
        b, h = divmod(core, 2)
        if h == 0:
            xo = xf[b]
        else:
            xo = np.concatenate([xf[b, NQ:], xf[b, :NQ]], 0)
        # channel-major transpose on host: [256, 4096] -> [128, 2, 4096]
        xT = np.ascontiguousarray(
            xo.T.reshape(2, 128, N).transpose(1, 0, 2).astype(BF16)
        )
        xq = np.ascontiguousarray(xo[:NQ])
        if use_bias:
            xq = xq + bo[None, :]
        im = {
            "xT": xT,
            "xq32": xq,
            "wblob": wblob,
        }
        if use_bias:
            im["wbias"] = wbias
        in_maps.append(im)
    return in_maps, use_bias


def _gather(results):
    out = np.empty((B, N, C), np.float32)
    for core in range(NCORES):
        b, h = divmod(core, 2)
        out[b, NQ * h : NQ * (h + 1)] = results[core]["out"]
    return out.reshape(B, HH, WW, C)


def kernel(x, wq, bq, wk, bk, wv, bv, wo, bo):
    from concourse.bass_utils import run_bass_kernel_spmd

    in_maps, use_bias = _prep(x, wq, bq, wk, bk, wv, bv, wo, bo)
    nc = _get_compiled(use_bias)
    res = run_bass_kernel_spmd(nc, in_maps, core_ids=list(range(NCORES)))
    return _gather(res.results)


def _ensure_ntff_hook():
    """The agent image's antenv stub lacks axon_hooks; synthesize it so
    run_bass_kernel_spmd(trace=True) can NTFF-profile via libaxon_pjrt."""
    import types

    try:
        from antenv.axon_hooks import get_axon_ntff_profile_hook  # noqa: F401
        return
    except ImportError:
        pass
    import antenv
    from trn_agent_boot.trn_boot import _ntff_profile_via_ctypes

    mod = types.ModuleType("antenv.axon_hooks")
    state = {"h": _ntff_profile_via_ctypes("/opt/axon/libaxon_pjrt.so")}
    mod.get_axon_ntff_profile_hook = lambda: state["h"]
    mod.set_axon_ntff_profile_hook = lambda h: state.__setitem__("h", h)
    sys.modules["antenv.axon_hooks"] = mod
    antenv.axon_hooks = mod


def run_traced(inputs, **kw):
    """For test.py: run with NTFF profiling; returns (output, BassKernelResults)."""
    from concourse.bass_utils import run_bass_kernel_spmd

    _ensure_ntff_hook()

    in_maps, use_bias = _prep(**inputs)
    nc = _get_compiled(use_bias)
    res = run_bass_kernel_spmd(nc, in_maps, core_ids=list(range(NCORES)), trace=True, **kw)
    return _gather(res.results), res


# revision 25
# speedup vs baseline: 1.1182x; 1.1182x over previous
"""Trainium2 Bass kernel for nn_AttentionBlock (B=4, H=W=64, C=256, D=32).

Sharding: 8 shards = 4 samples x 2 query-halves. Each core gets the full
sample (rows reordered so its 2048 query rows come first), computes K and
the fused V@Wo projection for all 4096 keys, and attention + residual for
its 2048 queries. No collectives needed.

Key structure (v4):
  - x is transposed on the host (layout prep), so no on-device transpose
    phase: xT arrives channel-major [128, 2, 4096].
  - wo is folded into the value projection on the host: W = x @ (wv@wo)
    (+ bv@wo). Attention output then IS the final projection:
    out = (exp(S) @ W) / denom + x_residual. No epilogue transpose or
    output-projection matmuls.
  - Scores (contraction D=32): q/k are replicated across the 4 partition
    bands (via column-replicated wq/wk), and the score matmul contracts
    over the FULL 128 partitions, computing 4x the dot product (the 1/4
    is folded into the host-side wq scale). Cost of a matmul is its
    free-dim column count (contraction-depth independent), so the
    redundancy is free; in exchange the PE never changes tile mode
    between score and attend matmuls (no array-drain penalties).
  - Softmax denominator via a ones-column appended to W (free dim 257).
  - Score PSUM [128, 1024] x 3 buffers (6 banks) so exp of superstep u
    overlaps score/attend matmuls of u+1, u+2 with no bank collisions.

Self-contained: hardcodes shapes, imports only /opt/trn_rl_repo concourse.
"""

import sys

if "/opt/trn_rl_repo" not in sys.path:
    sys.path.insert(0, "/opt/trn_rl_repo")

import numpy as np
import ml_dtypes

BF16 = ml_dtypes.bfloat16

# Problem constants
B, HH, WW, C = 4, 64, 64, 256
D = 32
N = HH * WW          # 4096 keys per sample
NQ = N // 2          # 2048 queries per core
NCORES = 8
KC = N // 128        # 32 key chunks

_compiled_cache = {}


def _build(use_bias: bool):
    from contextlib import ExitStack
    from concourse import bacc, tile, mybir

    f32 = mybir.dt.float32
    bf = mybir.dt.bfloat16
    f8 = mybir.dt.float8e4
    DR = mybir.MatmulPerfMode.DoubleRow

    nc = bacc.Bacc("TRN2", target_bir_lowering=False, debug=False, num_devices=NCORES)

    xT_d = nc.dram_tensor("xT", [128, 2, N], bf, kind="ExternalInput")
    xq32_d = nc.dram_tensor("xq32", [NQ, C], f32, kind="ExternalInput")
    # single blob: [wq0|wq1|wk0|wk1|wvo0|wvo1] so one DMA loads all weights
    wb_d = nc.dram_tensor("wblob", [128, 1024], bf, kind="ExternalInput")
    wbias_d = (
        nc.dram_tensor("wbias", [1, 512], bf, kind="ExternalInput")
        if use_bias
        else None
    )
    out_d = nc.dram_tensor("out", [NQ, C], f32, kind="ExternalOutput")

    Exp = mybir.ActivationFunctionType.Exp
    Add = mybir.AluOpType.add
    Mult = mybir.AluOpType.mult

    with tile.TileContext(nc) as tc:
        with ExitStack() as ctx:
            const = ctx.enter_context(tc.tile_pool(name="const", bufs=1))
            big = ctx.enter_context(tc.tile_pool(name="big", bufs=1))
            xbp = ctx.enter_context(tc.tile_pool(name="xbp", bufs=3))
            expp = ctx.enter_context(tc.tile_pool(name="expp", bufs=4))
            small = ctx.enter_context(tc.tile_pool(name="small", bufs=2))
            # PSUM: 3 x [128,1024] working tiles (6 banks; shared by phase-B
            # projections and score matmuls) + 2 pa accumulator banks = 8.
            ps_sc = ctx.enter_context(tc.tile_pool(name="ps_sc", bufs=3, space="PSUM"))
            ps_pa = ctx.enter_context(tc.tile_pool(name="ps_pa", bufs=2, space="PSUM"))

            # ---- weights (one blob DMA) ----
            wall = const.tile([128, 1024], bf, tag="wall")
            nc.sync.dma_start(out=wall[:], in_=wb_d[:])
            wq0 = wall[:, 0:128]
            wq1 = wall[:, 128:256]
            wk0 = wall[:, 256:384]
            wk1 = wall[:, 384:512]
            wvo0 = wall[:, 512:768]
            wvo1 = wall[:, 768:1024]
            if use_bias:
                ones_row = const.tile([1, 512], bf, tag="ones_row")
                nc.gpsimd.memset(ones_row[:], 1.0)
                wbias = const.tile([1, 512], bf, tag="wbias")
                nc.sync.dma_start(out=wbias[:], in_=wbias_d[:])
                wqb = wbias[:, 0:128]
                wkb = wbias[:, 128:256]
                wvob = wbias[:, 256:512]

            # Persistent SBUF: qT/kT replicated across the 4 partition bands
            # (so full-128-contraction score matmuls compute 4x the dot
            # product), and W (= V@Wo) rows with a ones column at 256 for the
            # softmax denominator.
            qT4 = big.tile([128, NQ], bf, tag="qT4")
            kT4 = big.tile([128, N], bf, tag="kT4")
            wsb = big.tile([128, 16, 2, 272], f8, tag="wsb")
            nc.vector.memset(wsb[:, :, :, 256:257], 1.0)
            expbias = const.tile([128, 1], f32, tag="expbias")
            nc.vector.memset(expbias[:], -2.0)

            # ---- phase B: DMA xT chunks, q/k proj (LDW-amortized), W proj ----
            xbs = []
            for s in range(8):
                xb = xbp.tile([128, 2, 512], bf, tag="xb", bufs=8)
                nc.sync.dma_start(out=xb[:], in_=xT_d[:, :, 512 * s : 512 * s + 512])
                xbs.append(xb)

            def qk_chunk(s):
                # q (chunks 0-3) and k share one [128,1024] psum tile
                p = ps_sc.tile([128, 1024], f32, tag="sc", name=f"pqk{s}")
                if s < 4:
                    nc.tensor.matmul(p[:, 0:512], wq0, xbs[s][:, 0, :], start=True, stop=False)
                    nc.tensor.matmul(p[:, 0:512], wq1, xbs[s][:, 1, :], start=False, stop=not use_bias)
                    if use_bias:
                        nc.tensor.matmul(p[:, 0:512], wqb, ones_row[:], start=False, stop=True)
                nc.tensor.matmul(p[:, 512:1024], wk0, xbs[s][:, 0, :], start=True, stop=False)
                nc.tensor.matmul(p[:, 512:1024], wk1, xbs[s][:, 1, :], start=False, stop=not use_bias)
                if use_bias:
                    nc.tensor.matmul(p[:, 512:1024], wkb, ones_row[:], start=False, stop=True)
                if s < 4:
                    nc.vector.tensor_copy(qT4[:, 512 * s : 512 * s + 512], p[:, 0:512])
                nc.vector.tensor_copy(kT4[:, 512 * s : 512 * s + 512], p[:, 512:1024])

            def w_pass(s):
                # W = x @ (wv@wo): 4 key chunks of 128 per xb in one psum tile
                pw = ps_sc.tile([128, 1024], f32, tag="sc", name=f"pw{s}")
                for j2 in range(4):
                    off = 128 * j2
                    nc.tensor.matmul(pw[:, 256 * j2 : 256 * j2 + 256], xbs[s][:, 0, off : off + 128], wvo0, start=True, stop=False)
                    nc.tensor.matmul(pw[:, 256 * j2 : 256 * j2 + 256], xbs[s][:, 1, off : off + 128], wvo1, start=False, stop=not use_bias)
                    if use_bias:
                        nc.tensor.matmul(pw[:, 256 * j2 : 256 * j2 + 256], ones_row[:, 0:128], wvob, start=False, stop=True)
                # evacuate on vector engine (scalar stays free for exps)
                nc.vector.tensor_copy(wsb[:, 2 * s : 2 * s + 2, :, 0:256], pw[:])

            # ---- phases B + C/D, software-pipelined ----
            # Phase B is emitted chunk-major and INTERLEAVED with group 0's
            # supersteps: superstep (0, u) only needs q chunk 0 and k/W of
            # chunk u, so the PE alternates projection and attention work
            # during the ramp and the scalar engine starts exp-ing early.
            # Superstep u of group g covers key chunks m = 4u + j. The score
            # matmul contracts over all 128 partitions (4 replicated bands of
            # q/k), computing 4x the true dot product; 1/4 is folded into the
            # host-side wq scale. Score psum [128, 1024] x bufs=3 (6 banks).

            def epilogue(qb, pa_t):
                rec = small.tile([128, 1], f32, tag="rec")
                nc.vector.reciprocal(rec[:], pa_t[:, 256:257])
                xq = small.tile([128, 256], f32, tag="xq", bufs=3)
                nc.sync.dma_start(out=xq[:], in_=xq32_d[128 * qb : 128 * qb + 128, :])
                sc = small.tile([128, 256], f32, tag="sc2")
                nc.vector.tensor_scalar(sc[:], pa_t[:, 0:256], rec[:], None, Mult)
                ot = small.tile([128, 256], f32, tag="ot", bufs=3)
                nc.vector.tensor_tensor(ot[:], sc[:], xq[:], Add)
                nc.sync.dma_start(out=out_d[128 * qb : 128 * qb + 128, :], in_=ot[:])

            pa_tiles = {}
            prev = None  # (et, g, u)
            NSS = 8 * 8  # 8 groups x 8 supersteps
            for idx in range(NSS + 1):
                if idx < 8:
                    # phase B chunk idx feeds superstep (0, idx)
                    qk_chunk(idx)
                    w_pass(idx)
                if idx < NSS:
                    g, u = divmod(idx, 8)
                    if u == 0:
                        pa_tiles[2 * g] = ps_pa.tile([128, 512], f32, tag="pa", name=f"pa{2 * g}")
                        pa_tiles[2 * g + 1] = ps_pa.tile([128, 512], f32, tag="pa", name=f"pa{2 * g + 1}")
                    pst = ps_sc.tile([128, 1024], f32, tag="sc")
                    for j in range(4):
                        m = 4 * u + j
                        nc.tensor.matmul(
                            pst[:, 256 * j : 256 * j + 256],
                            kT4[:, 128 * m : 128 * m + 128],
                            qT4[:, 256 * g : 256 * g + 256],
                            start=True,
                            stop=True,
                        )
                    # exp(s - 2): constant shift keeps exp within fp8-e4m3
                    # range (max score ~7 -> e^5 = 148 << 448; the tail below
                    # s = -4.2 flushes to zero, ~1e-7 of softmax mass). The
                    # ones-column denominator sees the same shift, so the
                    # normalized ratio is exact.
                    et = expp.tile([128, 2, 2, 256], f8, tag="e")
                    nc.scalar.activation(et[:], pst[:], Exp, bias=expbias[:])
                # attend with previous superstep's exp tile (keeps PE busy during exp)
                if prev is not None:
                    et_p, g_p, u_p = prev
                    for jp in range(2):
                        P = 2 * u_p + jp  # wsb pair: key chunks 2P, 2P+1
                        for h in range(2):
                            nc.tensor.matmul(
                                pa_tiles[2 * g_p + h][:, 0:257],
                                et_p[:, jp, :, 128 * h : 128 * h + 128],
                                wsb[:, P, :, 0:257],
                                start=(P == 0),
                                stop=(P == 15),
                                perf_mode=DR,
                            )
                    if u_p == 7:
                        for h in range(2):
                            epilogue(2 * g_p + h, pa_tiles[2 * g_p + h])
                            del pa_tiles[2 * g_p + h]
                if idx < NSS:
                    prev = (et, g, u)

    nc.compile()
    return nc


def _get_compiled(use_bias: bool):
    key = bool(use_bias)
    if key not in _compiled_cache:
        _compiled_cache[key] = _build(use_bias)
    return _compiled_cache[key]


def _prep(x, wq, bq, wk, bk, wv, bv, wo, bo):
    xf = np.ascontiguousarray(np.asarray(x, dtype=np.float32)).reshape(B, N, C)
    wq = np.asarray(wq, np.float32)
    bq = np.asarray(bq, np.float32)
    wk = np.asarray(wk, np.float32)
    bk = np.asarray(bk, np.float32)
    wv = np.asarray(wv, np.float32)
    bv = np.asarray(bv, np.float32)
    wo = np.asarray(wo, np.float32)
    bo = np.asarray(bo, np.float32)

    use_bias = not (
        np.all(bq == 0) and np.all(bk == 0) and np.all(bv == 0) and np.all(bo == 0)
    )

    # fold softmax scale into q, plus 1/4 because the score matmul contracts
    # over 4 replicated partition bands (summing the dot product 4x)
    scale = np.float32(1.0 / (4.0 * np.sqrt(np.float32(D))))
    wq_rep = np.tile(wq * scale, (1, 4)).astype(BF16)  # [256, 128]
    wk_rep = np.tile(wk, (1, 4)).astype(BF16)
    # fold wo into the value projection: W = x @ (wv@wo) + bv@wo
    wvo = (wv @ wo).astype(BF16)
    wblob = np.ascontiguousarray(
        np.concatenate(
            [wq_rep[0:128], wq_rep[128:256], wk_rep[0:128], wk_rep[128:256],
             wvo[0:128], wvo[128:256]],
            axis=1,
        )
    )  # [128, 1024]
    wbias = np.ascontiguousarray(
        np.concatenate(
            [np.tile(bq * scale, 4), np.tile(bk, 4), bv @ wo], 0
        )[None, :]
    ).astype(BF16)  # [1, 512]

    in_maps = []
    for core in range(NCORES):
        b, h = divmod(core, 2)
        if h == 0:
            xo = xf[b]
        else:
            xo = np.concatenate([xf[b, NQ:], xf[b, :NQ]], 0)
        # channel-major transpose on host: [256, 4096] -> [128, 2, 4096]
        xT = np.ascontiguousarray(
            xo.T.reshape(2, 128, N).transpose(1, 0, 2).astype(BF16)
        )
        xq = np.ascontiguousarray(xo[:NQ])
        if use_bias:
            xq = xq + bo[None, :]
        im = {
            "xT": xT,
            "xq32": xq,
            "wblob": wblob,
        }
        if use_bias:
            im["wbias"] = wbias
        in_maps.append(im)
    return in_maps, use_bias


def _gather(results):
    out = np.empty((B, N, C), np.float32)
    for core in range(NCORES):
        b, h = divmod(core, 2)
        out[b, NQ * h : NQ * (h + 1)] = results[core]["out"]
    return out.reshape(B, HH, WW, C)


def kernel(x, wq, bq, wk, bk, wv, bv, wo, bo):
    from concourse.bass_utils import run_bass_kernel_spmd

    in_maps, use_bias = _prep(x, wq, bq, wk, bk, wv, bv, wo, bo)
    nc = _get_compiled(use_bias)
    res = run_bass_kernel_spmd(nc, in_maps, core_ids=list(range(NCORES)))
    return _gather(res.results)


def _ensure_ntff_hook():
    """The agent image's antenv stub lacks axon_hooks; synthesize it so
    run_bass_kernel_spmd(trace=True) can NTFF-profile via libaxon_pjrt."""
    import types

    try:
        from antenv.axon_hooks import get_axon_ntff_profile_hook  # noqa: F401
        return
    except ImportError:
        pass
    import antenv
    from trn_agent_boot.trn_boot import _ntff_profile_via_ctypes

    mod = types.ModuleType("antenv.axon_hooks")
    state = {"h": _ntff_profile_via_ctypes("/opt/axon/libaxon_pjrt.so")}
    mod.get_axon_ntff_profile_hook = lambda: state["h"]
    mod.set_axon_ntff_profile_hook = lambda h: state.__setitem__("h", h)
    sys.modules["antenv.axon_hooks"] = mod
    antenv.axon_hooks = mod


def run_traced(inputs, **kw):
    """For test.py: run with NTFF profiling; returns (output, BassKernelResults)."""
    from concourse.bass_utils import run_bass_kernel_spmd

    _ensure_ntff_hook()

    in_maps, use_bias = _prep(**inputs)
    nc = _get_compiled(use_bias)
    res = run_bass_kernel_spmd(nc, in_maps, core_ids=list(range(NCORES)), trace=True, **kw)
    return _gather(res.results), res
